# revision 1
# baseline (speedup 1.0000x reference)
"""Fused CIN-layer kernel for Trainium2 (8 NeuronCores, batch data-parallel).

True reference semantics (derived from the row-major .view + strided conv):
  out[b, n, c*32+t] = sum_{i<32, y<32} W[n,i,y] * x0[b,t,2i+c] * xk[b,y,2i+c] + bias[n]
where c in {0,1} is the f-parity and i indexes f-pairs.

Per core (128 batches, bc = b_local*2 + c in [0,256), groups J of 4 bc's):
  stage1 (PE):  per i: G_i[n, bc] = sum_y W[n,i,y] * xk[b,y,2i+c]
                lhsT = Wst[:, i*64:(i+1)*64]  (32y x 64n)
                rhs  = XkS[:, i*256:(i+1)*256] (32y x 256bc)
                accumulated into [64, 1024] PSUM quads, ACT-evacuated to
                Gsb[n, i*256+bc] (fp16, contiguous).
  transpose (PE): per J: Gt_J[(j,i), n] = Gsb[n, (i, 4J+j)]^T via PE transpose
                with a strided input AP; DVE evacuates to SBUF fp16.
  stage2 (PE):  out_J[(j,t), n] = sum_{(j,i)} X0bd_J[(j,i),(j,t)] * Gt_J[(j,i),n]
                X0bd = host-built block-diagonal x0 tiles (fp16).
  ACT evacuates fp32 out; one big DMA out; host adds bias + final reshape.
"""

import numpy as np

BS, T, Y, F, NF = 1024, 32, 32, 64, 64
NCORES = 8
BPC = BS // NCORES      # 128 batches per core
NBC = BPC * 2           # 256 (b,c) pairs per core
NG = NBC // 4           # 64 groups of 4
NI = 32                 # f-pair index

_cached = {}


def _build_bass():
    import concourse.bass as bass
    import concourse.mybir as mybir
    from concourse import bacc
    from concourse.tile import TileContext

    F16 = mybir.dt.float16
    F32 = mybir.dt.float32

    nc = bacc.Bacc()
    xks = nc.dram_tensor("xks", [Y, NI * NBC], F16, kind="ExternalInput")
    wst = nc.dram_tensor("wst", [Y, NI * NF], F16, kind="ExternalInput")
    x0a = nc.dram_tensor("x0a", [128, NG * 128], F16, kind="ExternalInput")
    iden = nc.dram_tensor("iden", [NF, NF], F16, kind="ExternalInput")
    outd = nc.dram_tensor("outd", [128, NG * NF], F32, kind="ExternalOutput")

    with TileContext(nc) as tc:
        with (
            tc.tile_pool(name="const", bufs=1) as cpool,
            tc.tile_pool(name="sb", bufs=1) as spool,
            tc.tile_pool(name="gq", bufs=2, space="PSUM") as gqpool,
            tc.tile_pool(name="gt", bufs=2, space="PSUM") as gtpool,
            tc.tile_pool(name="po", bufs=2, space="PSUM") as popool,
        ):
            xks_sb = cpool.tile([Y, NI * NBC], F16)
            nc.sync.dma_start(out=xks_sb, in_=xks[:, :])
            wst_sb = cpool.tile([Y, NI * NF], F16)
            nc.sync.dma_start(out=wst_sb, in_=wst[:, :])
            x0a_sb = cpool.tile([128, NG * 128], F16)
            nc.sync.dma_start(out=x0a_sb, in_=x0a[:, :])
            id_sb = cpool.tile([NF, NF], F16)
            nc.sync.dma_start(out=id_sb, in_=iden[:, :])

            gsb = spool.tile([NF, NBC * NI], F16)   # G[n, bc*32+i]
            gt_sb = spool.tile([128, NG * NF], F16)  # Gt[(j,i), J*64+n]
            osb = spool.tile([128, NG * NF], F32)    # out[(j,t), J*64+n]

            # stage 1: 8 quads of 4 i-matmuls -> [64, 1024] psum -> Gsb
            for ii in range(NI // 4):
                gq = gqpool.tile([NF, 4 * NBC], mybir.dt.float32, tag="gq")
                for di in range(4):
                    i = 4 * ii + di
                    nc.tensor.matmul(
                        gq[:, di * NBC:(di + 1) * NBC],
                        wst_sb[:, i * NF:(i + 1) * NF],
                        xks_sb[:, i * NBC:(i + 1) * NBC],
                        start=True, stop=True,
                    )
                # interleaved evac: Gsb[n, bc*32 + i], i in this quad
                # split by bc-half across ACT and DVE to halve drain latency
                in_ap = gq[:, :].rearrange("p (di bc) -> p bc di", di=4, bc=NBC)
                out_ap = gsb[:, :].rearrange(
                    "p (bc i) -> p bc i", bc=NBC, i=NI)[:, :, 4 * ii:4 * ii + 4]
                h = NBC // 2
                nc.scalar.copy(out_ap[:, :h], in_ap[:, :h])
                nc.vector.tensor_copy(out_ap[:, h:], in_ap[:, h:])

            # transpose: per J, contiguous read of Gsb -> Gt[(j,i), n]
            for J8 in range(NG // 8):
                gt8 = gtpool.tile([128, 8 * NF], F16, tag="gt8")
                for s in range(8):
                    J = J8 * 8 + s
                    nc.tensor.transpose(
                        gt8[:, s * NF:(s + 1) * NF],
                        gsb[:, J * 128:(J + 1) * 128],
                        id_sb[:, :],
                    )
                nc.vector.tensor_copy(
                    gt_sb[:, J8 * 8 * NF:(J8 + 1) * 8 * NF], gt8[:, :])

            # stage 2: per J, block-diag x0 matmul
            for J8 in range(NG // 8):
                po = popool.tile([128, 8 * NF], mybir.dt.float32, tag="po")
                for s in range(8):
                    J = J8 * 8 + s
                    nc.tensor.matmul(
                        po[:, s * NF:(s + 1) * NF],
                        x0a_sb[:, J * 128:(J + 1) * 128],
                        gt_sb[:, J * NF:(J + 1) * NF],
                        start=True, stop=True,
                    )
                o0 = J8 * 8 * NF
                hh = 4 * NF
                nc.scalar.copy(osb[:, o0:o0 + hh], po[:, :hh])
                nc.vector.tensor_copy(osb[:, o0 + hh:o0 + 8 * NF], po[:, hh:])
                nc.sync.dma_start(out=outd[:, o0:o0 + 8 * NF],
                                  in_=osb[:, o0:o0 + 8 * NF])
    nc.compile()
    return nc


def _host_prep(x_0, x_k, weight):
    f16 = np.float16
    x_0 = np.asarray(x_0, dtype=np.float32)
    x_k = np.asarray(x_k, dtype=np.float32)
    W = np.asarray(weight, dtype=np.float32).reshape(NF, NI, Y)

    # Wst[y, i*64+n] = W[n, i, y]
    wst = np.ascontiguousarray(W.transpose(2, 1, 0).reshape(Y, NI * NF)).astype(f16)

    xks_l, x0a_l = [], []
    jj = np.arange(4)
    for core in range(NCORES):
        xkc = x_k[core * BPC:(core + 1) * BPC]            # [128, y, f]
        x0c = x_0[core * BPC:(core + 1) * BPC]            # [128, t, f]
        # XkS[y, i*256 + b_l*2 + c] = xk[b_l, y, 2i+c]
        xkr = xkc.reshape(BPC, Y, NI, 2)                  # [b_l, y, i, c]
        xks = xkr.transpose(1, 2, 0, 3).reshape(Y, NI * NBC)
        xks_l.append(np.ascontiguousarray(xks).astype(f16))
        # x0 per bc: [bc, i, t]
        x0r = x0c.reshape(BPC, T, NI, 2)                  # [b_l, t, i, c]
        x0bc = x0r.transpose(0, 3, 2, 1).reshape(NBC, NI, T)
        # block-diagonal tiles: X0bd[J, j, i, j2, t] = delta(j,j2)*x0bc[4J+j, i, t]
        x0bd = np.zeros((NG, 4, NI, 4, T), dtype=np.float32)
        x0bd[:, jj, :, jj, :] = x0bc.reshape(NG, 4, NI, T).transpose(1, 0, 2, 3)
        # rows (j, i), cols (J, j2, t)
        x0a = x0bd.transpose(1, 2, 0, 3, 4).reshape(128, NG * 128)
        x0a_l.append(np.ascontiguousarray(x0a).astype(f16))

    iden = np.eye(NF, dtype=np.float32).astype(f16)
    return xks_l, x0a_l, wst, iden


def kernel(x_0, x_k, weight, bias):
    from concourse import bass_utils

    if "nc" not in _cached:
        _cached["nc"] = _build_bass()
    nc = _cached["nc"]

    xks_l, x0a_l, wst, iden = _host_prep(x_0, x_k, weight)
    in_maps = [
        {"xks": xks_l[c], "x0a": x0a_l[c], "wst": wst, "iden": iden}
        for c in range(NCORES)
    ]
    res = bass_utils.run_bass_kernel_spmd(nc, in_maps, core_ids=list(range(NCORES)))

    bias = np.asarray(bias, dtype=np.float32)
    outs = []
    for c in range(NCORES):
        od = res.results[c]["outd"]                 # [128=(j,t), NG*64=(J,n)]
        o = od.reshape(4, T, NG, NF)                # [j, t, J, n]
        o = o.transpose(2, 0, 3, 1)                 # [J, j, n, t]
        o = o.reshape(BPC, 2, NF, T)                # [b_l, c, n, t]
        o = o.transpose(0, 2, 1, 3).reshape(BPC, NF, 2 * T)  # [b_l, n, c*32+t]
        outs.append(o)
    out = np.concatenate(outs, axis=0)
    out = out + bias[None, :, None]
    return np.ascontiguousarray(out.astype(np.float32))



# revision 9
# speedup vs baseline: 1.0856x; 1.0856x over previous
"""Fused CIN-layer kernel for Trainium2 (8 NeuronCores, batch data-parallel).

True reference semantics (derived from the row-major .view + strided conv):
  out[b, n, c*32+t] = sum_{i<32, y<32} W[n,i,y] * x0[b,t,2i+c] * xk[b,y,2i+c] + bias[n]
where c in {0,1} is the f-parity and i indexes f-pairs.

Per core (128 batches, bc = b_local*2 + c in [0,256)):
  warmup:  ~14 dummy matmuls on a zeroed tile while input DMA streams, so
           the PE HAM clock-gate reaches 2.4 GHz before the real work.
  stage1:  row-tiled quads: for quad q, 4 concurrent K=32 matmuls (one per
           i=4q+di, tile_position=(32*di,0)) with lhsT=W_i [32y,64n] and
           rhs=XkS_i [32y,256bc], each into its own PSUM bank of a
           [64, 2048] quad tile.  3-way ACT/DVE/GPSIMD evac into
           gsb[n, bc*32+i] (fp16).
  transpose (PE): per J (4 bc's): Gt_J[(j,i), n] from gsb[n, (j,i)] via PE
           transpose (fp16 PSUM out), DVE-evacuated to SBUF.
  stage2:  out_J[(j,t), n] = x0bd_J^T @ Gt_J where x0bd is a block-diagonal
           x0 tile built ON DEVICE (memset + 4 strided scatter DMAs of the
           compact 512KB x0), loaded as stationary with FWL.
  fp16 output DMA per J8 chunk; host adds bias + final reshape in fp32.
"""

import numpy as np

BS, T, Y, F, NF = 1024, 32, 32, 64, 64
NCORES = 8
BPC = BS // NCORES      # 128 batches per core
NBC = BPC * 2           # 256 (b,c) pairs per core
NG = NBC // 4           # 64 groups of 4
NI = 32                 # f-pair index
NWARM = 14              # HAM warmup matmuls

_cached = {}


def _build_bass():
    import concourse.bass as bass
    import concourse.mybir as mybir
    from concourse import bacc
    from concourse.tile import TileContext

    F16 = mybir.dt.float16
    F32 = mybir.dt.float32

    nc = bacc.Bacc()
    # xks4[di*32+y, q*256+bc] = xk[b, y, 2*(4q+di)+c],  bc = 2*b_l + c
    xks4 = nc.dram_tensor("xks4", [128, 8 * NBC], F16, kind="ExternalInput")
    # w4[di*32+y, q*64+n] = W[n, 4q+di, y]
    w4 = nc.dram_tensor("w4", [128, 8 * NF], F16, kind="ExternalInput")
    # x0a[(j,i), J*128 + j2*32 + t] = delta(j,j2) * x0bc[4J+j, i, t]
    x0a = nc.dram_tensor("x0a", [128, NG * 128], F16, kind="ExternalInput")
    iden = nc.dram_tensor("iden", [NF, NF], F16, kind="ExternalInput")
    # outd[(j,t), J*64+n] fp16
    outd = nc.dram_tensor("outd", [128, NG * NF], F16, kind="ExternalOutput")

    with TileContext(nc) as tc:
        with (
            tc.tile_pool(name="const", bufs=1) as cpool,
            tc.tile_pool(name="sb", bufs=1) as spool,
        ):
            wdum = cpool.tile([128, 384], F16)
            nc.vector.memset(wdum[:, :], 0)
            # dependency-free scalar op so ACT_TABLE_LOAD happens during the
            # preamble instead of stalling the first stage-1 evacuation
            scratch = cpool.tile([64, 8], F16)
            nc.scalar.copy(scratch[:, :], wdum[0:64, 0:8])

            xks_sb = cpool.tile([128, 8 * NBC], F16)
            nc.sync.dma_start(out=xks_sb, in_=xks4[:, :])
            w4_sb = cpool.tile([128, 8 * NF], F16)
            nc.sync.dma_start(out=w4_sb, in_=w4[:, :])
            id_sb = cpool.tile([NF, NF], F16)
            nc.sync.dma_start(out=id_sb, in_=iden[:, :])
            x0a_sb = cpool.tile([128, NG * 128], F16)
            nc.sync.dma_start(out=x0a_sb, in_=x0a[:, :])

            gsb = spool.tile([NF, NBC * NI], F16)    # G[n, bc*32+i]
            osb = spool.tile([128, NG * NF], F16)    # out[(j,t), J*64+n]

            # HAM warmup: harmless matmuls on zeros while inputs stream in
            with tc.tile_pool(name="warm", bufs=1, space="PSUM") as wpool:
                wt = wpool.tile([128, 256], F32)
                for _ in range(NWARM):
                    nc.tensor.matmul(wt[:, :], wdum[:, :128], wdum[:, 128:384],
                                     start=True, stop=True)

            # stage 1: 8 quads, each 4 row-tiled concurrent matmuls
            with tc.tile_pool(name="gq", bufs=2, space="PSUM") as gqpool:
                for q in range(8):
                    gq = gqpool.tile([NF, 2048], F32, tag="gq")
                    for di in range(4):
                        nc.tensor.matmul(
                            gq[:, 512 * di:512 * di + NBC],
                            w4_sb[32 * di:32 * (di + 1), NF * q:NF * (q + 1)],
                            xks_sb[32 * di:32 * (di + 1), NBC * q:NBC * (q + 1)],
                            start=True, stop=True,
                            tile_position=(32 * di, 0),
                        )
                    # evac interleave: gsb[n, bc*32 + (4q+di)], 3-way split by bc
                    src = gq[:, :].rearrange("p (di b) -> p b di", di=4, b=512)[:, :NBC, :]
                    dst = gsb[:, :].rearrange("p (b i) -> p b i", b=NBC, i=NI)[:, :, 4 * q:4 * q + 4]
                    nc.vector.tensor_copy(dst[:, 0:144], src[:, 0:144])
                    nc.scalar.copy(dst[:, 144:256], src[:, 144:256])

            # transpose + stage2, software-pipelined per J8 round
            with (
                tc.tile_pool(name="gt", bufs=2, space="PSUM") as gtpool,
                tc.tile_pool(name="po", bufs=2, space="PSUM") as popool,
                tc.tile_pool(name="gts", bufs=2) as gtspool,
            ):
                gts_tiles = {}

                def do_tr(J8):
                    gt8 = gtpool.tile([128, 8 * NF], F16, tag="gt8")
                    for s in range(8):
                        J = 8 * J8 + s
                        nc.tensor.transpose(
                            gt8[:, NF * s:NF * (s + 1)],
                            gsb[:, 128 * J:128 * (J + 1)],
                            id_sb[:, :],
                        )
                    gts = gtspool.tile([128, 8 * NF], F16, tag="gts")
                    nc.vector.tensor_copy(gts[:, :], gt8[:, :])
                    gts_tiles[J8] = gts

                def do_s2(J8):
                    gts = gts_tiles.pop(J8)
                    po = popool.tile([128, 8 * NF], F32, tag="po")
                    for s in range(8):
                        J = 8 * J8 + s
                        nc.tensor.matmul(
                            po[:, NF * s:NF * (s + 1)],
                            x0a_sb[:, 128 * J:128 * (J + 1)],
                            gts[:, NF * s:NF * (s + 1)],
                            start=True, stop=True,
                        )
                    o0 = J8 * 8 * NF
                    if J8 % 2 == 0:
                        nc.scalar.copy(osb[:, o0:o0 + 8 * NF], po[:, :])
                    else:
                        nc.vector.tensor_copy(osb[:, o0:o0 + 8 * NF], po[:, :])
                    nc.sync.dma_start(out=outd[:, o0:o0 + 8 * NF],
                                      in_=osb[:, o0:o0 + 8 * NF])

                do_tr(0)
                for J8 in range(1, 8):
                    do_tr(J8)
                    do_s2(J8 - 1)
                do_s2(7)
    nc.compile()
    return nc


def _host_prep(x_0, x_k, weight):
    f16 = np.float16
    x_0 = np.asarray(x_0, dtype=np.float32)
    x_k = np.asarray(x_k, dtype=np.float32)
    W = np.asarray(weight, dtype=np.float32).reshape(NF, NI, Y)

    # w4[di*32+y, q*64+n] = W[n, 4q+di, y]
    w4 = np.ascontiguousarray(
        W.reshape(NF, 8, 4, Y).transpose(2, 3, 1, 0).reshape(128, 8 * NF)
    ).astype(f16)

    xks_l, x0a_l = [], []
    jj = np.arange(4)
    for core in range(NCORES):
        xkc = x_k[core * BPC:(core + 1) * BPC]            # [128, y, f]
        x0c_ = x_0[core * BPC:(core + 1) * BPC]           # [128, t, f]
        # xks4[di*32+y, q*256 + b_l*2 + c] = xk[b_l, y, 2*(4q+di)+c]
        xkr = xkc.reshape(BPC, Y, 8, 4, 2)                # [b_l, y, q, di, c]
        xks4 = xkr.transpose(3, 1, 2, 0, 4).reshape(128, 8 * NBC)
        xks_l.append(np.ascontiguousarray(xks4).astype(f16))
        # x0a[(j,i), (J,j2,t)] = delta(j,j2) * x0bc[4J+j, i, t]
        x0r = x0c_.reshape(BPC, T, NI, 2)                 # [b_l, t, i, c]
        x0bc = x0r.transpose(0, 3, 2, 1).reshape(NBC, NI, T)   # [bc, i, t]
        x0bd = np.zeros((NG, 4, NI, 4, T), dtype=np.float32)
        x0bd[:, jj, :, jj, :] = x0bc.reshape(NG, 4, NI, T).transpose(1, 0, 2, 3)
        x0a = x0bd.transpose(1, 2, 0, 3, 4).reshape(128, NG * 128)
        x0a_l.append(np.ascontiguousarray(x0a).astype(f16))

    iden = np.eye(NF, dtype=np.float32).astype(f16)
    return xks_l, x0a_l, w4, iden


def _in_maps(x_0, x_k, weight):
    xks_l, x0a_l, w4, iden = _host_prep(x_0, x_k, weight)
    return [
        {"xks4": xks_l[c], "x0a": x0a_l[c], "w4": w4, "iden": iden}
        for c in range(NCORES)
    ]


def kernel(x_0, x_k, weight, bias):
    from concourse import bass_utils

    if "nc" not in _cached:
        _cached["nc"] = _build_bass()
    nc = _cached["nc"]

    in_maps = _in_maps(x_0, x_k, weight)
    res = bass_utils.run_bass_kernel_spmd(nc, in_maps, core_ids=list(range(NCORES)))

    bias = np.asarray(bias, dtype=np.float32)
    outs = []
    for c in range(NCORES):
        od = res.results[c]["outd"].astype(np.float32)  # [128=(j,t), NG*64=(J,n)]
        o = od.reshape(4, T, NG, NF)                # [j, t, J, n]
        o = o.transpose(2, 0, 3, 1)                 # [J, j, n, t]
        o = o.reshape(BPC, 2, NF, T)                # [b_l, c, n, t]
        o = o.transpose(0, 2, 1, 3).reshape(BPC, NF, 2 * T)  # [b_l, n, c*32+t]
        outs.append(o)
    out = np.concatenate(outs, axis=0)
    out = out + bias[None, :, None]
    return np.ascontiguousarray(out.astype(np.float32))


# revision 14
# speedup vs baseline: 1.2432x; 1.1452x over previous
"""Fused CIN-layer kernel for Trainium2 (8 NeuronCores, batch data-parallel).

True reference semantics (derived from the row-major .view + strided conv):
  out[b, n, c*32+t] = sum_{i<32, y<32} W[n,i,y] * x0[b,t,2i+c] * xk[b,y,2i+c] + bias[n]
where c in {0,1} is the f-parity and i indexes f-pairs.

Per core (128 batches, bc = b_local*2 + c in [0,256)):
  warmup:  ~14 dummy matmuls on a zeroed tile while input DMA streams, so
           the PE HAM clock-gate reaches 2.4 GHz before the real work.
  stage1:  row-tiled quads: for quad q, 4 concurrent K=32 matmuls (one per
           i=4q+di, tile_position=(32*di,0)) with lhsT=W_i [32y,64n] and
           rhs=XkS_i [32y,256bc], each into its own PSUM bank of a
           [64, 2048] quad tile.  3-way ACT/DVE/GPSIMD evac into
           gsb[n, bc*32+i] (fp16).
  transpose (PE): per J (4 bc's): Gt_J[(j,i), n] from gsb[n, (j,i)] via PE
           transpose (fp16 PSUM out), DVE-evacuated to SBUF.
  stage2:  out_J[(j,t), n] = x0bd_J^T @ Gt_J where x0bd is a block-diagonal
           x0 tile built ON DEVICE (memset + 4 strided scatter DMAs of the
           compact 512KB x0), loaded as stationary with FWL.
  fp16 output DMA per J8 chunk; host adds bias + final reshape in fp32.
"""

import numpy as np

BS, T, Y, F, NF = 1024, 32, 32, 64, 64
NCORES = 8
BPC = BS // NCORES      # 128 batches per core
NBC = BPC * 2           # 256 (b,c) pairs per core
NG = NBC // 4           # 64 groups of 4
NI = 32                 # f-pair index
NWARM = 8               # HAM warmup matmuls

_cached = {}


def _build_bass():
    import concourse.bass as bass
    import concourse.mybir as mybir
    from concourse import bacc
    from concourse.tile import TileContext

    F16 = mybir.dt.float16
    F32 = mybir.dt.float32

    nc = bacc.Bacc()
    # xks4[di*32+y, q*256+bc] = xk[b, y, 2*(4q+di)+c],  bc = 2*b_l + c
    xks4 = nc.dram_tensor("xks4", [128, 8 * NBC], F16, kind="ExternalInput")
    # w4[di*32+y, q*64+n] = W[n, 4q+di, y]
    w4 = nc.dram_tensor("w4", [128, 8 * NF], F16, kind="ExternalInput")
    # x0a[(j,i), J*128 + j2*32 + t] = delta(j,j2) * x0bc[4J+j, i, t]
    x0a = nc.dram_tensor("x0a", [128, NG * 128], F16, kind="ExternalInput")
    iden = nc.dram_tensor("iden", [NF, NF], F16, kind="ExternalInput")
    # outd[(j,t), J*64+n] fp16
    outd = nc.dram_tensor("outd", [128, NG * NF], F16, kind="ExternalOutput")

    with TileContext(nc) as tc:
        with (
            tc.tile_pool(name="const", bufs=1) as cpool,
            tc.tile_pool(name="sb", bufs=1) as spool,
        ):
            wdum = cpool.tile([128, 384], F16)
            nc.vector.memset(wdum[:, :], 0)
            # dependency-free scalar op so ACT_TABLE_LOAD happens during the
            # preamble instead of stalling the first stage-1 evacuation
            scratch = cpool.tile([64, 8], F16)
            nc.scalar.copy(scratch[:, :], wdum[0:64, 0:8])

            w4_sb = cpool.tile([128, 8 * NF], F16)
            nc.sync.dma_start(out=w4_sb, in_=w4[:, :])
            # xks in two tiles so early quads can start before the full DMA
            xks_sb = [cpool.tile([128, 4 * NBC], F16, name=f"xks_sb{h}") for h in range(2)]
            for h in range(2):
                nc.sync.dma_start(out=xks_sb[h],
                                  in_=xks4[:, 4 * NBC * h:4 * NBC * (h + 1)])
            id_sb = cpool.tile([NF, NF], F16)
            nc.sync.dma_start(out=id_sb, in_=iden[:, :])
            # x0a in two tiles so early s2 rounds aren't gated on the full 2MB
            x0a_sb = [cpool.tile([128, NG * 64], F16, name=f"x0a_sb{h}") for h in range(2)]
            for h in range(2):
                nc.sync.dma_start(out=x0a_sb[h],
                                  in_=x0a[:, NG * 64 * h:NG * 64 * (h + 1)])

            gsb = spool.tile([NF, NBC * NI], F16)    # G[n, bc*32+i]
            osb = spool.tile([128, NG * NF], F16)    # out[(j,t), J*64+n]

            # HAM warmup: harmless matmuls on zeros while inputs stream in
            with tc.tile_pool(name="warm", bufs=1, space="PSUM") as wpool:
                wt = wpool.tile([128, 256], F32)
                for _ in range(NWARM):
                    nc.tensor.matmul(wt[:, :], wdum[:, :128], wdum[:, 128:384],
                                     start=True, stop=True)

            # stage 1: 8 quads, each 4 row-tiled concurrent matmuls.
            # Each quad uses two 2-bank psum tiles (di pair each) so the
            # evacuations (DVE on one, ACT on the other) pipeline finely.
            with tc.tile_pool(name="gq", bufs=4, space="PSUM") as gqpool:
                for q in range(8):
                    xq = xks_sb[q // 4]
                    qc = q % 4
                    gqt = [gqpool.tile([NF, 1024], F32, tag="gq", name=f"gq_{q}_{h}") for h in range(2)]
                    for di in range(4):
                        nc.tensor.matmul(
                            gqt[di // 2][:, 512 * (di % 2):512 * (di % 2) + NBC],
                            w4_sb[32 * di:32 * (di + 1), NF * q:NF * (q + 1)],
                            xq[32 * di:32 * (di + 1), NBC * qc:NBC * (qc + 1)],
                            start=True, stop=True,
                            tile_position=(32 * di, 0),
                        )
                    # evac interleave: gsb[n, bc*32 + (4q+di)]
                    dstq = gsb[:, :].rearrange("p (b i) -> p b i", b=NBC, i=NI)
                    for h in range(2):
                        src = gqt[h][:, :].rearrange(
                            "p (di b) -> p b di", di=2, b=512)[:, :NBC, :]
                        dst = dstq[:, :, 4 * q + 2 * h:4 * q + 2 * h + 2]
                        if h == 0:
                            nc.vector.tensor_copy(dst, src)
                        else:
                            nc.scalar.copy(dst, src)

            # transpose + stage2, software-pipelined per J8 round
            with (
                tc.tile_pool(name="gt", bufs=2, space="PSUM") as gtpool,
                tc.tile_pool(name="po", bufs=2, space="PSUM") as popool,
                tc.tile_pool(name="gts", bufs=2) as gtspool,
            ):
                gts_tiles = {}

                def do_tr(J8):
                    gt8 = gtpool.tile([128, 8 * NF], F16, tag="gt8")
                    for s in range(8):
                        J = 8 * J8 + s
                        nc.tensor.transpose(
                            gt8[:, NF * s:NF * (s + 1)],
                            gsb[:, 128 * J:128 * (J + 1)],
                            id_sb[:, :],
                        )
                    gts = gtspool.tile([128, 8 * NF], F16, tag="gts")
                    nc.vector.tensor_copy(gts[:, :], gt8[:, :])
                    gts_tiles[J8] = gts

                def do_s2(J8):
                    gts = gts_tiles.pop(J8)
                    po = popool.tile([128, 8 * NF], F32, tag="po")
                    x0h = x0a_sb[J8 // 4]
                    for s in range(8):
                        J = 8 * (J8 % 4) + s
                        nc.tensor.matmul(
                            po[:, NF * s:NF * (s + 1)],
                            x0h[:, 128 * J:128 * (J + 1)],
                            gts[:, NF * s:NF * (s + 1)],
                            start=True, stop=True,
                        )
                    o0 = J8 * 8 * NF
                    if J8 % 2 == 0:
                        nc.scalar.copy(osb[:, o0:o0 + 8 * NF], po[:, :])
                    else:
                        nc.vector.tensor_copy(osb[:, o0:o0 + 8 * NF], po[:, :])
                    nc.sync.dma_start(out=outd[:, o0:o0 + 8 * NF],
                                      in_=osb[:, o0:o0 + 8 * NF])

                do_tr(0)
                for J8 in range(1, 8):
                    do_tr(J8)
                    do_s2(J8 - 1)
                do_s2(7)
    nc.compile()
    return nc


def _host_prep(x_0, x_k, weight):
    f16 = np.float16
    x_0 = np.asarray(x_0, dtype=np.float32)
    x_k = np.asarray(x_k, dtype=np.float32)
    W = np.asarray(weight, dtype=np.float32).reshape(NF, NI, Y)

    # w4[di*32+y, q*64+n] = W[n, 4q+di, y]
    w4 = np.ascontiguousarray(
        W.reshape(NF, 8, 4, Y).transpose(2, 3, 1, 0).reshape(128, 8 * NF)
    ).astype(f16)

    xks_l, x0a_l = [], []
    jj = np.arange(4)
    for core in range(NCORES):
        xkc = x_k[core * BPC:(core + 1) * BPC]            # [128, y, f]
        x0c_ = x_0[core * BPC:(core + 1) * BPC]           # [128, t, f]
        # xks4[di*32+y, q*256 + b_l*2 + c] = xk[b_l, y, 2*(4q+di)+c]
        xkr = xkc.reshape(BPC, Y, 8, 4, 2)                # [b_l, y, q, di, c]
        xks4 = xkr.transpose(3, 1, 2, 0, 4).reshape(128, 8 * NBC)
        xks_l.append(np.ascontiguousarray(xks4).astype(f16))
        # x0a[(j,i), (J,j2,t)] = delta(j,j2) * x0bc[4J+j, i, t]
        x0r = x0c_.reshape(BPC, T, NI, 2)                 # [b_l, t, i, c]
        x0bc = x0r.transpose(0, 3, 2, 1).reshape(NBC, NI, T)   # [bc, i, t]
        x0bd = np.zeros((NG, 4, NI, 4, T), dtype=np.float32)
        x0bd[:, jj, :, jj, :] = x0bc.reshape(NG, 4, NI, T).transpose(1, 0, 2, 3)
        x0a = x0bd.transpose(1, 2, 0, 3, 4).reshape(128, NG * 128)
        x0a_l.append(np.ascontiguousarray(x0a).astype(f16))

    iden = np.eye(NF, dtype=np.float32).astype(f16)
    return xks_l, x0a_l, w4, iden


def _in_maps(x_0, x_k, weight):
    xks_l, x0a_l, w4, iden = _host_prep(x_0, x_k, weight)
    return [
        {"xks4": xks_l[c], "x0a": x0a_l[c], "w4": w4, "iden": iden}
        for c in range(NCORES)
    ]


def kernel(x_0, x_k, weight, bias):
    from concourse import bass_utils

    if "nc" not in _cached:
        _cached["nc"] = _build_bass()
    nc = _cached["nc"]

    in_maps = _in_maps(x_0, x_k, weight)
    res = bass_utils.run_bass_kernel_spmd(nc, in_maps, core_ids=list(range(NCORES)))

    bias = np.asarray(bias, dtype=np.float32)
    outs = []
    for c in range(NCORES):
        od = res.results[c]["outd"].astype(np.float32)  # [128=(j,t), NG*64=(J,n)]
        o = od.reshape(4, T, NG, NF)                # [j, t, J, n]
        o = o.transpose(2, 0, 3, 1)                 # [J, j, n, t]
        o = o.reshape(BPC, 2, NF, T)                # [b_l, c, n, t]
        o = o.transpose(0, 2, 1, 3).reshape(BPC, NF, 2 * T)  # [b_l, n, c*32+t]
        outs.append(o)
    out = np.concatenate(outs, axis=0)
    out = out + bias[None, :, None]
    return np.ascontiguousarray(out.astype(np.float32))


# revision 23
# speedup vs baseline: 1.3014x; 1.0468x over previous
"""Fused CIN-layer kernel for Trainium2 (8 NeuronCores, batch data-parallel).

True reference semantics (derived from the row-major .view + strided conv):
  out[b, n, c*32+t] = sum_{i<32, y<32} W[n,i,y] * x0[b,t,2i+c] * xk[b,y,2i+c] + bias[n]
where c in {0,1} is the f-parity and i indexes f-pairs.

Per core (128 batches, bc = b_local*2 + c in [0,256)):
  warmup:  ~14 dummy matmuls on a zeroed tile while input DMA streams, so
           the PE HAM clock-gate reaches 2.4 GHz before the real work.
  stage1:  row-tiled quads: for quad q, 4 concurrent K=32 matmuls (one per
           i=4q+di, tile_position=(32*di,0)) with lhsT=W_i [32y,64n] and
           rhs=XkS_i [32y,256bc], each into its own PSUM bank of a
           [64, 2048] quad tile.  3-way ACT/DVE/GPSIMD evac into
           gsb[n, bc*32+i] (fp16).
  transpose (PE): per J (4 bc's): Gt_J[(j,i), n] from gsb[n, (j,i)] via PE
           transpose (fp16 PSUM out), DVE-evacuated to SBUF.
  stage2:  out_J[(j,t), n] = x0bd_J^T @ Gt_J where x0bd is a block-diagonal
           x0 tile built ON DEVICE (memset + 4 strided scatter DMAs of the
           compact 512KB x0), loaded as stationary with FWL.
  fp16 output DMA per J8 chunk; host adds bias + final reshape in fp32.
"""

import numpy as np

BS, T, Y, F, NF = 1024, 32, 32, 64, 64
NCORES = 8
BPC = BS // NCORES      # 128 batches per core
NBC = BPC * 2           # 256 (b,c) pairs per core
NG = NBC // 4           # 64 groups of 4
NI = 32                 # f-pair index
NWARM = 8               # HAM warmup matmuls

_cached = {}


def _build_bass():
    import concourse.bass as bass
    import concourse.mybir as mybir
    from concourse import bacc
    from concourse.tile import TileContext

    F16 = mybir.dt.float16
    F32 = mybir.dt.float32

    nc = bacc.Bacc()
    # comba[di*32+y | (n,i)-rows, :]: cols 0:512   w4[., q*64+n]  = W[n,4q+di,y]
    #                                cols 512:1536 xks half0 [., q*256+bc]
    #                                cols 1536:1600 iden (rows 0:64)
    comba = nc.dram_tensor("comba", [128, 1600], F16, kind="ExternalInput")
    # xksb: xks half1 (quads 4..7)
    xksb = nc.dram_tensor("xksb", [128, 4 * NBC], F16, kind="ExternalInput")
    # x0a[(j,i), J*128 + j2*32 + t] = delta(j,j2) * x0bc[4J+j, i, t]
    x0a = nc.dram_tensor("x0a", [128, NG * 128], F16, kind="ExternalInput")
    # outd[(j,t), J*64+n] fp16
    outd = nc.dram_tensor("outd", [128, NG * NF], F16, kind="ExternalOutput")

    with TileContext(nc) as tc:
        with (
            tc.tile_pool(name="const", bufs=1) as cpool,
            tc.tile_pool(name="sb", bufs=1) as spool,
        ):
            wdum = cpool.tile([128, 384], F16)
            nc.vector.memset(wdum[:, :], 0)
            # dependency-free scalar op so ACT_TABLE_LOAD happens during the
            # preamble instead of stalling the first stage-1 evacuation
            scratch = cpool.tile([64, 8], F16)
            nc.scalar.copy(scratch[:, :], wdum[0:64, 0:8])

            # single DMA for everything stage-1 needs (w4 + xks half0 + iden)
            comba_sb = cpool.tile([128, 1600], F16)
            nc.sync.dma_start(out=comba_sb, in_=comba[:, :])
            xksb_sb = cpool.tile([128, 4 * NBC], F16)
            nc.sync.dma_start(out=xksb_sb, in_=xksb[:, :])

            def w4_ap(di, q):
                return comba_sb[32 * di:32 * (di + 1), NF * q:NF * (q + 1)]

            def xks_ap(di, q):
                qc = q % 4
                if q < 4:
                    return comba_sb[32 * di:32 * (di + 1),
                                    512 + NBC * qc:512 + NBC * (qc + 1)]
                return xksb_sb[32 * di:32 * (di + 1), NBC * qc:NBC * (qc + 1)]

            id_sb = comba_sb[0:NF, 1536:1600]
            # x0a in two tiles so early s2 rounds aren't gated on the full 2MB
            x0a_sb = [cpool.tile([128, NG * 64], F16, name=f"x0a_sb{h}") for h in range(2)]
            for h in range(2):
                nc.sync.dma_start(out=x0a_sb[h],
                                  in_=x0a[:, NG * 64 * h:NG * 64 * (h + 1)])

            gsb = spool.tile([NF, NBC * NI], F16)    # G[n, bc*32+i]
            osb = spool.tile([128, NG * NF], F16)    # out[(j,t), J*64+n]

            # HAM warmup: harmless matmuls on zeros while inputs stream in
            with tc.tile_pool(name="warm", bufs=1, space="PSUM") as wpool:
                wt = wpool.tile([128, 256], F32)
                for _ in range(NWARM):
                    nc.tensor.matmul(wt[:, :], wdum[:, :128], wdum[:, 128:384],
                                     start=True, stop=True)

            # stage 1: 8 quads, each 4 row-tiled concurrent matmuls.
            # Each quad uses two 2-bank psum tiles (di pair each) so the
            # evacuations (DVE on one, ACT on the other) pipeline finely.
            with tc.tile_pool(name="gq", bufs=4, space="PSUM") as gqpool:
                for q in range(8):
                    gqt = [gqpool.tile([NF, 1024], F32, tag="gq", name=f"gq_{q}_{h}") for h in range(2)]
                    for di in range(4):
                        nc.tensor.matmul(
                            gqt[di // 2][:, 512 * (di % 2):512 * (di % 2) + NBC],
                            w4_ap(di, q),
                            xks_ap(di, q),
                            start=True, stop=True,
                            tile_position=(32 * di, 0),
                        )
                    # evac interleave: gsb[n, bc*32 + (4q+di)]
                    dstq = gsb[:, :].rearrange("p (b i) -> p b i", b=NBC, i=NI)
                    for h in range(2):
                        src = gqt[h][:, :].rearrange(
                            "p (di b) -> p b di", di=2, b=512)[:, :NBC, :]
                        dst = dstq[:, :, 4 * q + 2 * h:4 * q + 2 * h + 2]
                        if h == 0:
                            nc.vector.tensor_copy(dst, src)
                        else:
                            nc.scalar.copy(dst, src)

            # transpose + stage2, software-pipelined per J8 round
            with (
                tc.tile_pool(name="gt", bufs=2, space="PSUM") as gtpool,
                tc.tile_pool(name="po", bufs=2, space="PSUM") as popool,
                tc.tile_pool(name="gts", bufs=2) as gtspool,
            ):
                gts_tiles = {}

                def do_tr(J8):
                    gt8 = gtpool.tile([128, 8 * NF], F16, tag="gt8")
                    for s in range(8):
                        J = 8 * J8 + s
                        nc.tensor.transpose(
                            gt8[:, NF * s:NF * (s + 1)],
                            gsb[:, 128 * J:128 * (J + 1)],
                            id_sb,
                        )
                    gts = gtspool.tile([128, 8 * NF], F16, tag="gts")
                    nc.vector.tensor_copy(gts[:, :], gt8[:, :])
                    gts_tiles[J8] = gts

                def do_s2(J8):
                    gts = gts_tiles.pop(J8)
                    po = popool.tile([128, 8 * NF], F32, tag="po")
                    x0h = x0a_sb[J8 // 4]
                    for s in range(8):
                        J = 8 * (J8 % 4) + s
                        nc.tensor.matmul(
                            po[:, NF * s:NF * (s + 1)],
                            x0h[:, 128 * J:128 * (J + 1)],
                            gts[:, NF * s:NF * (s + 1)],
                            start=True, stop=True,
                        )
                    o0 = J8 * 8 * NF
                    if J8 % 2 == 0:
                        nc.scalar.copy(osb[:, o0:o0 + 8 * NF], po[:, :])
                    else:
                        nc.vector.tensor_copy(osb[:, o0:o0 + 8 * NF], po[:, :])
                        # one output DMA per pair of J8 rounds (fewer, bigger
                        # issues -- DMA_DIRECT2D issue cost dominates the tail)
                        d0 = (J8 - 1) * 8 * NF
                        nc.sync.dma_start(out=outd[:, d0:d0 + 16 * NF],
                                          in_=osb[:, d0:d0 + 16 * NF])

                do_tr(0)
                for J8 in range(1, 8):
                    do_tr(J8)
                    do_s2(J8 - 1)
                do_s2(7)
    nc.compile()
    return nc


def _host_prep(x_0, x_k, weight):
    f16 = np.float16
    x_0 = np.asarray(x_0, dtype=np.float32)
    x_k = np.asarray(x_k, dtype=np.float32)
    W = np.asarray(weight, dtype=np.float32).reshape(NF, NI, Y)

    # w4[di*32+y, q*64+n] = W[n, 4q+di, y]
    w4f = W.reshape(NF, 8, 4, Y).transpose(2, 3, 1, 0).reshape(128, 8 * NF)

    iden = np.eye(NF, dtype=np.float32)

    comba_l, xksb_l, x0a_l = [], [], []
    jj = np.arange(4)
    for core in range(NCORES):
        xkc = x_k[core * BPC:(core + 1) * BPC]            # [128, y, f]
        x0c_ = x_0[core * BPC:(core + 1) * BPC]           # [128, t, f]
        # xks4[di*32+y, q*256 + b_l*2 + c] = xk[b_l, y, 2*(4q+di)+c]
        xkr = xkc.reshape(BPC, Y, 8, 4, 2)                # [b_l, y, q, di, c]
        xks4 = xkr.transpose(3, 1, 2, 0, 4).reshape(128, 8 * NBC)
        comba = np.zeros((128, 1600), dtype=np.float32)
        comba[:, 0:512] = w4f
        comba[:, 512:1536] = xks4[:, :4 * NBC]
        comba[0:NF, 1536:1600] = iden
        comba_l.append(comba.astype(f16))
        xksb_l.append(np.ascontiguousarray(xks4[:, 4 * NBC:]).astype(f16))
        # x0a[(j,i), (J,j2,t)] = delta(j,j2) * x0bc[4J+j, i, t]
        x0r = x0c_.reshape(BPC, T, NI, 2)                 # [b_l, t, i, c]
        x0bc = x0r.transpose(0, 3, 2, 1).reshape(NBC, NI, T)   # [bc, i, t]
        x0bd = np.zeros((NG, 4, NI, 4, T), dtype=np.float32)
        x0bd[:, jj, :, jj, :] = x0bc.reshape(NG, 4, NI, T).transpose(1, 0, 2, 3)
        x0a = x0bd.transpose(1, 2, 0, 3, 4).reshape(128, NG * 128)
        x0a_l.append(np.ascontiguousarray(x0a).astype(f16))

    return comba_l, xksb_l, x0a_l


def _in_maps(x_0, x_k, weight):
    comba_l, xksb_l, x0a_l = _host_prep(x_0, x_k, weight)
    return [
        {"comba": comba_l[c], "xksb": xksb_l[c], "x0a": x0a_l[c]}
        for c in range(NCORES)
    ]


def kernel(x_0, x_k, weight, bias):
    from concourse import bass_utils

    if "nc" not in _cached:
        _cached["nc"] = _build_bass()
    nc = _cached["nc"]

    in_maps = _in_maps(x_0, x_k, weight)
    res = bass_utils.run_bass_kernel_spmd(nc, in_maps, core_ids=list(range(NCORES)))

    bias = np.asarray(bias, dtype=np.float32)
    outs = []
    for c in range(NCORES):
        od = res.results[c]["outd"].astype(np.float32)  # [128=(j,t), NG*64=(J,n)]
        o = od.reshape(4, T, NG, NF)                # [j, t, J, n]
        o = o.transpose(2, 0, 3, 1)                 # [J, j, n, t]
        o = o.reshape(BPC, 2, NF, T)                # [b_l, c, n, t]
        o = o.transpose(0, 2, 1, 3).reshape(BPC, NF, 2 * T)  # [b_l, n, c*32+t]
        outs.append(o)
    out = np.concatenate(outs, axis=0)
    out = out + bias[None, :, None]
    return np.ascontiguousarray(out.astype(np.float32))
